# revision 14
# baseline (speedup 1.0000x reference)
"""Trainium2 Bass kernel for an AttnBlock (LayerNorm -> qkv -> feature-axis
attention -> proj -> residual), sharded batch-parallel across 8 NeuronCores.

Self-contained: hardcodes shapes (B=8, L=4096, D=1024, H=1) and runs via
concourse run_bass_kernel_spmd on cores 0-7.

Math per batch element b (n = b since H == 1):
    h   = LayerNorm(x) * norm_w + norm_b
    qkv = h @ qkv_w.T + qkv_b            # [L, 3D]
    q, k, v = qkv[:, :D], qkv[:, D:2D], qkv[:, 2D:]
    S   = q.T @ (k / sqrt(L))            # [D, D]  (contract over L)
    W   = softmax(S, axis=1)
    A   = v @ W.T                        # [L, D]
    out = A @ proj_w.T + proj_b + x

Fast path (zero qkv/norm biases, which setup_inputs always produces): the
q/k/v GEMMs are algebraically collapsed through the Gram matrix G = h.T @ h:
    S   = Wq.T @ G @ Wk                  (no q,k materialization)
    out_attn = h @ (Wv @ (Wn.T @ projT)) (no v,A materialization; Wn = softmax)
This halves the matmul FLOPs (51.6 -> 25.6 GFLOP per core) and removes the
W-transpose and all HBM spills. All matmuls bf16 with fp32 PSUM accumulation.

Phases (per core):
    A:  stream x in L-chunks of 128: LN -> h (bf16, kept fully in SBUF),
        G = h.T h accumulated in PSUM in two 4-row-tile passes.
    B:  T1 = G @ Wk      (G used as lhsT via symmetry)
    C:  S = Wq.T @ T1; softmax per 128-row tile (exp w/ scale 1/sqrt(L),
        accum rowsum), Wn = W / rowsum (bf16)
    D:  MT = Wn.T @ projT
    E2: N = Wv @ MT
    F:  per L-chunk: PE-transpose h chunk, out = hT.T @ N + (x + proj_b)

A general path (the original direct kernel, handling nonzero biases) is kept
as fallback.
"""

import math
import re
from contextlib import ExitStack

import ml_dtypes
import numpy as np

import concourse.bass as bass
import concourse.mybir as mybir
import concourse.tile as tile
from concourse.vector_clock import ScopedClock, VectorClock

F32 = mybir.dt.float32
BF16 = mybir.dt.bfloat16
AF = mybir.ActivationFunctionType
ALU = mybir.AluOpType

P = 128
D = 1024
NKT = D // P  # 8 tiles over D
LN_EPS = 1e-5


def _vc_ticks(vc):
    return [int(s) for s in re.findall(r"\d+", repr(vc))]


def _patched_drain_and_barrier(self, tick_clock, wait_clock):
    # This walrus build rejects >1 sync wait on one CTRL instruction; split
    # the kernel-tail drain into one drain per busy logical processor.
    for proc, t in enumerate(_vc_ticks(tick_clock.global_clock)):
        if t <= 0:
            continue
        d = self.nc.sync.drain()
        sub = VectorClock()
        sub.require_at_least(proc, t)
        wait_clock.add_sem_waits(d.ins, ScopedClock({None: sub}))
    self.nc.all_engine_barrier()
    popped = self.nc._tile_sem_poison_stack.pop()
    assert popped is self._sem_poison
    self.nc.clear_and_free_semaphores(list(self.sems.allocated().values()))
    self.nc.all_engine_barrier()


tile.TileContext._drain_and_barrier = _patched_drain_and_barrier

# This walrus build rejects >1 sync wait on any instruction. Spill excess
# waits onto preceding single-wait NoOps on the same engine (program order
# on the engine stream makes the split equivalent).
_MAXW = 1
_orig_commit = tile.TileContext._commit_instruction


def _commit_capped(self, inst, lazy_reg_writes=True):
    si = getattr(inst, "sync_info", None)
    eng = getattr(inst, "engine", None)
    if (si is not None and si.on_wait and len(si.on_wait) > _MAXW
            and eng is not None and eng != mybir.EngineType.Unassigned):
        waits = list(si.on_wait)
        while len(waits) > _MAXW:
            chunk, waits = waits[:_MAXW], waits[_MAXW:]
            nop = mybir.InstNoOp(
                name=f"I-{self.nc.next_id()}",
                sync_info=mybir.SyncInfo(on_wait=chunk, on_update=[]),
                bass_nofuse=True,
                engine=eng,
            )
            _orig_commit(self, nop, lazy_reg_writes=False)
        inst.sync_info = mybir.SyncInfo(on_wait=waits, on_update=si.on_update)
    return _orig_commit(self, inst, lazy_reg_writes)


tile.TileContext._commit_instruction = _commit_capped


# --------------------------------------------------------------------------
# Gram-restructured fast path (zero qkv/norm biases)
# --------------------------------------------------------------------------

def build_program_gram(L):
    NL = L // P  # 32 L-chunks
    nc = bass.Bass("TRN2", target_bir_lowering=False, debug=False)

    x_d = nc.dram_tensor("x", [L, D], F32, kind="ExternalInput").ap()
    xres_d = nc.dram_tensor("xres", [L, D], F32, kind="ExternalInput").ap()
    wq_d = nc.dram_tensor("wqT", [D, D], BF16, kind="ExternalInput").ap()
    wk_d = nc.dram_tensor("wkT", [D, D], BF16, kind="ExternalInput").ap()
    wv_d = nc.dram_tensor("wvKD", [D, D], BF16, kind="ExternalInput").ap()
    proj_d = nc.dram_tensor("projT", [D, D], BF16, kind="ExternalInput").ap()
    ident_d = nc.dram_tensor("ident", [P, P], BF16, kind="ExternalInput").ap()
    out_d = nc.dram_tensor("out", [L, D], F32, kind="ExternalOutput").ap()

    with tile.TileContext(nc) as tc:
        _emit_gram(tc, L, NL, x_d, xres_d, wq_d, wk_d, wv_d, proj_d, ident_d,
                   out_d)
    return nc


def _emit_gram(tc, L, NL, x_d, xres_d, wq_d, wk_d, wv_d, proj_d, ident_d,
               out_d):
    nc = tc.nc
    inv_sqrt_l = float(1.0 / math.sqrt(L))

    with ExitStack() as octx:
        # The tile allocator is a per-space stack: pools must be released in
        # LIFO order of their first allocation, so persistent tiles are
        # allocated up-front ordered by (reverse) release time. Two scratch
        # buffers are double-used across phases (T1 then MT; G then Wn) to
        # fund a deep x-stream pipeline. Peak SBUF ~193KB/partition.
        const = octx.enter_context(tc.tile_pool(name="const", bufs=1))
        ident = const.tile([P, P], BF16)
        eps_t = const.tile([P, 1], F32)

        hpool = octx.enter_context(tc.tile_pool(name="h", bufs=1))
        h_sb = hpool.tile([P, NL, D], BF16)

        npl = octx.enter_context(tc.tile_pool(name="nsb", bufs=1))
        n_sb = npl.tile([P, NKT, D], BF16)

        sE2 = ExitStack()   # projT/wv weights, T1/MT scratch, BE psum
        sD = ExitStack()    # G/Wn scratch
        sC = ExitStack()    # wqk (wk then wq), softmax scalars
        sA = ExitStack()    # x stream, LN stats, G psum

        w2 = sE2.enter_context(tc.tile_pool(name="w2", bufs=1))
        proj_sb = w2.tile([P, NKT, D], BF16)
        wv_sb = w2.tile([P, NKT, D], BF16)
        s1p = sE2.enter_context(tc.tile_pool(name="scr1", bufs=1))
        scr1 = s1p.tile([P, NKT, D], BF16)      # T1 in B..C, MT in D..E2

        s2p = sD.enter_context(tc.tile_pool(name="scr2", bufs=1))
        scr2 = s2p.tile([P, NKT, D], BF16)      # G in A..B, Wn in C..D

        w1 = sC.enter_context(tc.tile_pool(name="w1", bufs=1))
        wqk_sb = w1.tile([P, NKT, D], BF16)     # Wk in B, Wq in C
        sxp = sC.enter_context(tc.tile_pool(name="sxp", bufs=4))

        NPRE = 10
        xin = sA.enter_context(tc.tile_pool(name="xin", bufs=NPRE))
        x_pre = {}
        for c in range(min(NPRE, NL)):
            x_pre[c] = xin.tile([P, D], F32, tag="x0", name=f"xp{c}")
        stp = sA.enter_context(tc.tile_pool(name="stats", bufs=6))

        # DMA emission order: x prefetch first (sync queue), weights behind
        # it on the idle gpsimd queue as per-tile contiguous copies (the
        # fused rearrange DMA costs ~5-6us of issue time on the sequencer).
        for c in range(min(NPRE, NL)):
            nc.sync.dma_start(out=x_pre[c][:], in_=x_d[c * P:(c + 1) * P, :])
        nc.vector.memset(eps_t[:], LN_EPS)
        nc.gpsimd.dma_start(out=ident[:], in_=ident_d[:])
        wk_view = wk_d.rearrange("(t p) n -> p t n", p=P)
        wq_view = wq_d.rearrange("(t p) n -> p t n", p=P)
        proj_view = proj_d.rearrange("(t p) n -> p t n", p=P)
        wv_view = wv_d.rearrange("(t p) n -> p t n", p=P)
        # Weight loads ride the gpsimd queue, which also carries the tiny
        # per-chunk nmr op: issue them INTERLEAVED with the chunk stream
        # (one per chunk) so they never head-of-line block the LN chain.
        # Wq is staged in n_sb's space (N is only written in E2, long after
        # phase C's last Wq read) - avoids a wk->wq reload seam before C.
        wdma_q = (
            [(wqk_sb, wk_view, t) for t in range(NKT)]
            + [(n_sb, wq_view, t) for t in range(NKT)]
            + [(proj_sb, proj_view, t) for t in range(NKT)]
            + [(wv_sb, wv_view, t) for t in range(NKT)]
        )

        # ---------------- Phase A: LN + h + G = h.T h ----------------
        # LN is split into front (stats, on DVE+scalar) and back (normalize)
        # halves, software-pipelined two chunks apart so DVE's in-order queue
        # never head-of-line blocks on the scalar sqrt. nmr rides GpSimd.
        def ln_front(c):
            if c in x_pre:
                xt = x_pre.pop(c)
            else:
                xt = xin.tile([P, D], F32, tag="x0", name=f"x{c}")
                nc.sync.dma_start(out=xt[:], in_=x_d[c * P:(c + 1) * P, :])
            st = stp.tile([P, 2, 6], F32, name=f"st{c}", tag="st")
            nc.vector.bn_stats(out=st[:, 0, :], in_=xt[:, 0:512])
            nc.vector.bn_stats(out=st[:, 1, :], in_=xt[:, 512:D])
            mv_t = stp.tile([P, 2], F32, name=f"mv{c}", tag="mv")
            nc.vector.bn_aggr(out=mv_t[:], in_=st[:])
            rstd = stp.tile([P, 1], F32, name=f"rstd{c}", tag="rstd")
            nc.scalar.activation(
                out=rstd[:], in_=mv_t[:, 1:2], func=AF.Sqrt,
                bias=eps_t[:], scale=1.0)
            return xt, mv_t, rstd

        def ln_back(c, xt, mv_t, rstd):
            nc.vector.reciprocal(out=rstd[:], in_=rstd[:])
            nmr = stp.tile([P, 1], F32, name=f"nmr{c}", tag="nmr")
            nc.gpsimd.tensor_scalar(
                out=nmr[:], in0=mv_t[:, 0:1], scalar1=rstd[:],
                scalar2=-1.0, op0=ALU.mult, op1=ALU.mult)
            nc.scalar.activation(
                out=h_sb[:, c, :], in_=xt[:], func=AF.Identity,
                scale=rstd[:], bias=nmr[:])
            # drip one weight-tile DMA issue per chunk behind the nmr
            if wdma_q:
                dst, view, t = wdma_q.pop(0)
                nc.gpsimd.dma_start(out=dst[:, t, :], in_=view[:, t, :])
            if c >= NL - 4:
                while wdma_q:
                    dst, view, t = wdma_q.pop(0)
                    nc.gpsimd.dma_start(out=dst[:, t, :], in_=view[:, t, :])

        # G is symmetric: compute only the upper triangle (row-tile m covers
        # cols >= m*128) and fill the lower triangle with PE transposes.
        # Pass 0 (rows 0-3, 8 padded PSUM banks) overlaps the x stream;
        # pass 1 (rows 4-7, 4 banks) re-reads h from SBUF and overlaps the
        # lower-triangle fill-in.
        def g_row_mms(ptile, c, m):
            lhs = h_sb[:, c, m * P:(m + 1) * P]
            base = m * P
            rem = D - base
            off = 0
            while rem > 0:
                w = min(512, rem)
                nc.tensor.matmul(
                    ptile[:, off:off + w], lhs,
                    h_sb[:, c, base + off:base + off + w],
                    start=(c == 0), stop=(c == NL - 1))
                off += w
                rem -= w

        sGa = ExitStack()
        pGa = sGa.enter_context(tc.tile_pool(name="pGa", bufs=1, space="PSUM"))
        ptiles = [pGa.tile([P, D], F32, name=f"ga{m}", tag=f"g{m}")
                  for m in range(4)]
        pend = {}
        for c in range(min(2, NL)):
            pend[c] = ln_front(c)
        for c in range(NL):
            if c + 2 < NL:
                pend[c + 2] = ln_front(c + 2)
            ln_back(c, *pend.pop(c))
            for m in range(4):
                g_row_mms(ptiles[m], c, m)
        for m in range(4):
            nc.scalar.copy(out=scr2[:, m, m * P:D],
                           in_=ptiles[m][:, :D - m * P])
        sA.close()   # frees x-stream SBUF
        sGa.close()  # frees the 8 pass-0 PSUM banks

        sGb = ExitStack()
        pGb = sGb.enter_context(
            tc.tile_pool(name="pGb", bufs=1, space="PSUM"))
        pFill = sGb.enter_context(
            tc.tile_pool(name="pFill", bufs=2, space="PSUM"))

        def fill_row(r, cs):
            # scr2[:, r, c-slice] = transpose of scr2[:, c, r-slice]
            for c0 in range(0, len(cs), 4):
                grp = cs[c0:c0 + 4]
                pt = pFill.tile([P, 512], F32, name=f"fl{r}_{c0}", tag="fl")
                for j, c in enumerate(grp):
                    nc.tensor.matmul(
                        pt[:, j * P:(j + 1) * P],
                        scr2[:, c, r * P:(r + 1) * P], ident[:],
                        start=True, stop=True)
                nc.scalar.copy(
                    out=scr2[:, r, grp[0] * P:(grp[0] + len(grp)) * P],
                    in_=pt[:, :len(grp) * P].rearrange(
                        "p (j c2) -> p j c2", j=len(grp)))

        # pass 1 first (depends only on h), then the lower-triangle fills,
        # high columns first to match the descending-m T1 loop in phase B.
        gtiles = [pGb.tile([P, 512], F32, name=f"gb{m}", tag=f"gb{m}")
                  for m in range(4, 8)]
        for c in range(NL):
            for m in range(4, 8):
                g_row_mms(gtiles[m - 4], c, m)
        for m in range(4, 8):
            nc.scalar.copy(out=scr2[:, m, m * P:D],
                           in_=gtiles[m - 4][:, :D - m * P])
        for r in range(7, 4, -1):
            fill_row(r, list(range(4, r)))
        for r in range(7, 3, -1):
            fill_row(r, list(range(4)))
        for r in range(3, 0, -1):
            fill_row(r, list(range(r)))
        sGb.close()

        # ---------------- Phase B: T1 = G @ Wk ----------------
        psBE = sE2.enter_context(
            tc.tile_pool(name="psBE", bufs=3, space="PSUM"))

        for m in reversed(range(NKT)):
            pt1 = psBE.tile([P, D], F32, name=f"t1_{m}", tag="ps")
            for t in range(NKT):
                # lhsT = G[:, t, m-slice]: by symmetry of G this is the
                # [K=d2, M=d1] operand for row-block m of T1.
                lhs = scr2[:, t, m * P:(m + 1) * P]
                for nn in range(2):
                    nc.tensor.matmul(
                        pt1[:, nn * 512:(nn + 1) * 512], lhs,
                        wqk_sb[:, t, nn * 512:(nn + 1) * 512],
                        start=(t == 0), stop=(t == NKT - 1))
            nc.scalar.copy(out=scr1[:, m, :], in_=pt1[:])

        # ---------------- Phase C: S = Wq.T @ T1, softmax -> Wn --------
        for qs in range(NKT):
            ps = psBE.tile([P, D], F32, name=f"s_{qs}", tag="ps")
            for t in range(NKT):
                lhs = n_sb[:, t, qs * P:(qs + 1) * P]
                for nn in range(2):
                    nc.tensor.matmul(
                        ps[:, nn * 512:(nn + 1) * 512], lhs,
                        scr1[:, t, nn * 512:(nn + 1) * 512],
                        start=(t == 0), stop=(t == NKT - 1))
            # softmax (no max-subtraction: S = q.T k / sqrt(L) is O(5) for
            # normalized inputs; exp stays in fp32/bf16 range). exp writes
            # the G/Wn scratch directly; normalization is an in-place scale.
            sumexp = sxp.tile([P, 1], F32, name=f"se{qs}", tag="se")
            nc.scalar.activation(
                out=scr2[:, qs, :], in_=ps[:], func=AF.Exp, bias=0.0,
                scale=inv_sqrt_l, accum_out=sumexp[:])
            rsr = sxp.tile([P, 1], F32, name=f"rs{qs}", tag="rs")
            nc.vector.reciprocal(out=rsr[:], in_=sumexp[:])
            nc.vector.tensor_scalar_mul(
                out=scr2[:, qs, :], in0=scr2[:, qs, :], scalar1=rsr[:])

        sC.close()

        # ---------------- Phase D: MT = Wn.T @ projT ----------------
        for kt in range(NKT):
            pmt = psBE.tile([P, D], F32, name=f"mt_{kt}", tag="ps")
            for qs in range(NKT):
                lhs = scr2[:, qs, kt * P:(kt + 1) * P]
                for nn in range(2):
                    nc.tensor.matmul(
                        pmt[:, nn * 512:(nn + 1) * 512], lhs,
                        proj_sb[:, qs, nn * 512:(nn + 1) * 512],
                        start=(qs == 0), stop=(qs == NKT - 1))
            nc.scalar.copy(out=scr1[:, kt, :], in_=pmt[:])

        sD.close()

        # ---------------- Phase E2: N = Wv @ MT ----------------
        for dt in range(NKT):
            pn = psBE.tile([P, D], F32, name=f"n_{dt}", tag="ps")
            for kt in range(NKT):
                lhs = wv_sb[:, kt, dt * P:(dt + 1) * P]
                for nn in range(2):
                    nc.tensor.matmul(
                        pn[:, nn * 512:(nn + 1) * 512], lhs,
                        scr1[:, kt, nn * 512:(nn + 1) * 512],
                        start=(kt == 0), stop=(kt == NKT - 1))
            nc.scalar.copy(out=n_sb[:, dt, :], in_=pn[:])

        sE2.close()

        # ---------------- Phase F: out = h @ N + xres ----------------
        # Software-pipelined: chunk c+1's PE transposes are emitted before
        # chunk c's out matmuls so the PE never waits on the hT copy.
        htp = octx.enter_context(tc.tile_pool(name="hTc", bufs=3))
        ptp = octx.enter_context(
            tc.tile_pool(name="ptrans", bufs=2, space="PSUM"))
        pop = octx.enter_context(tc.tile_pool(name="po", bufs=2, space="PSUM"))
        xrp = octx.enter_context(tc.tile_pool(name="xr", bufs=4))
        osp = octx.enter_context(tc.tile_pool(name="ost", bufs=3))

        def transpose_chunk(c):
            xr = xrp.tile([P, D], F32, name=f"xr{c}", tag="xr")
            nc.sync.dma_start(out=xr[:], in_=xres_d[c * P:(c + 1) * P, :])
            hTc = htp.tile([P, NKT, P], BF16, name=f"hT{c}", tag="hT")
            for jh in range(2):
                pt = ptp.tile([P, 512], F32, name=f"pt{c}_{jh}", tag="pt")
                for jj in range(4):
                    j = jh * 4 + jj
                    nc.tensor.matmul(
                        pt[:, jj * P:(jj + 1) * P],
                        h_sb[:, c, j * P:(j + 1) * P], ident[:],
                        start=True, stop=True)
                nc.scalar.copy(
                    out=hTc[:, jh * 4:(jh + 1) * 4, :],
                    in_=pt[:].rearrange("p (j c2) -> p j c2", j=4))
            return xr, hTc

        cur = transpose_chunk(0)
        for c in range(NL):
            nxt = transpose_chunk(c + 1) if c + 1 < NL else None
            xr, hTc = cur
            po = pop.tile([P, D], F32, name=f"po{c}", tag="po")
            for t in range(NKT):
                for nn in range(2):
                    nc.tensor.matmul(
                        po[:, nn * 512:(nn + 1) * 512], hTc[:, t, :],
                        n_sb[:, t, nn * 512:(nn + 1) * 512],
                        start=(t == 0), stop=(t == NKT - 1))
            o_sb = osp.tile([P, D], F32, name=f"o{c}", tag="o")
            nc.vector.tensor_add(out=o_sb[:], in0=po[:], in1=xr[:])
            nc.sync.dma_start(
                out=out_d[c * P:(c + 1) * P, :], in_=o_sb[:])
            cur = nxt


def make_in_map_gram(xb, qkv_w, norm_w, proj_w, proj_b):
    qkv_w = np.asarray(qkv_w, np.float32)
    norm_w = np.asarray(norm_w, np.float32)
    wfold = qkv_w * norm_w[None, :]
    bf = ml_dtypes.bfloat16
    return {
        "x": np.ascontiguousarray(xb, np.float32),
        "xres": (np.asarray(xb, np.float32)
                 + np.asarray(proj_b, np.float32)[None, :]),
        "wqT": np.ascontiguousarray(wfold[:D].T).astype(bf),
        "wkT": np.ascontiguousarray(wfold[D:2 * D].T).astype(bf),
        "wvKD": np.ascontiguousarray(wfold[2 * D:]).astype(bf),
        "projT": np.ascontiguousarray(
            np.asarray(proj_w, np.float32).T).astype(bf),
        "ident": np.eye(P, dtype=bf),
    }


# --------------------------------------------------------------------------
# General fallback path (nonzero qkv/norm biases): original direct kernel
# --------------------------------------------------------------------------

def build_program_general(L, zero_bias=True):
    NL = L // P  # number of 128-row L chunks
    NG = L // 512  # number of 512-row L groups
    nc = bass.Bass("TRN2", target_bir_lowering=False, debug=False)

    x_d = nc.dram_tensor("x", [L, D], F32, kind="ExternalInput").ap()
    xres_d = nc.dram_tensor("xres", [L, D], F32, kind="ExternalInput").ap()
    wqk_d = nc.dram_tensor("wqkT", [D, 2 * D], BF16, kind="ExternalInput").ap()
    wv_d = nc.dram_tensor("wvT", [D, D], BF16, kind="ExternalInput").ap()
    proj_d = nc.dram_tensor("projT", [D, D], BF16, kind="ExternalInput").ap()
    biasqk_d = nc.dram_tensor("biasqk", [2 * D], F32, kind="ExternalInput").ap()
    biasv_d = nc.dram_tensor("biasv", [D], F32, kind="ExternalInput").ap()
    ident_d = nc.dram_tensor("ident", [P, P], BF16, kind="ExternalInput").ap()
    out_d = nc.dram_tensor("out", [L, D], F32, kind="ExternalOutput").ap()

    k_spill = nc.dram_tensor("k_spill", [L, D], BF16).ap()
    vt_spill = nc.dram_tensor("vt_spill", [D, L], BF16).ap()

    with tile.TileContext(nc) as tc:
        _emit_general(tc, L, NL, NG, x_d, xres_d, wqk_d, wv_d, proj_d,
                      biasqk_d, biasv_d, ident_d, out_d, k_spill, vt_spill,
                      zero_bias)
    return nc


def _emit_general(tc, L, NL, NG, x_d, xres_d, wqk_d, wv_d, proj_d, biasqk_d,
                  biasv_d, ident_d, out_d, k_spill, vt_spill, zero_bias):
    nc = tc.nc

    with ExitStack() as octx:
        const = octx.enter_context(tc.tile_pool(name="const", bufs=1))
        ident = const.tile([P, P], BF16)
        nc.sync.dma_start(out=ident[:], in_=ident_d[:])
        eps_t = const.tile([P, 1], F32)
        nc.vector.memset(eps_t[:], LN_EPS)
        proj_sb = const.tile([P, NKT, D], BF16)
        if not zero_bias:
            biasqk = const.tile([P, 2 * D], F32)
            nc.sync.dma_start(
                out=biasqk[:], in_=biasqk_d[None, :].to_broadcast((P, 2 * D)))
            biasv = const.tile([P, NKT], F32)
            nc.sync.dma_start(
                out=biasv[:], in_=biasv_d.rearrange("(mv p) -> p mv", p=P))
        # per-q-tile softmax 1/rowsum, filled in phase C, consumed in D
        rs_sb = const.tile([P, NKT], F32)

        qpool = octx.enter_context(tc.tile_pool(name="qres", bufs=1))
        q_sb = qpool.tile([P, NL, D], BF16)

        # ---------------- Phase AB: LN + qkv projection ----------------
        with ExitStack() as ab:
            xin = ab.enter_context(tc.tile_pool(name="xin", bufs=6))
            x_pre = {}
            for c in range(min(6, NL)):
                x_pre[c] = xin.tile([P, D], F32, tag="x0", name=f"xp{c}")
                nc.sync.dma_start(out=x_pre[c][:],
                                  in_=x_d[c * P:(c + 1) * P, :])

            abw = ab.enter_context(tc.tile_pool(name="abw", bufs=1))
            wqk = abw.tile([P, NKT, 2 * D], BF16)
            wqk_view = wqk_d.rearrange("(kt p) n -> p kt n", p=P)
            for kt in range(NKT):
                nc.sync.dma_start(out=wqk[:, kt, :], in_=wqk_view[:, kt, :])
            wv = abw.tile([P, NKT, D], BF16)
            nc.sync.dma_start(
                out=wv[:], in_=wv_d.rearrange("(kt p) n -> p kt n", p=P))

            stp = ab.enter_context(tc.tile_pool(name="stats", bufs=3))
            hp = ab.enter_context(tc.tile_pool(name="h", bufs=3))
            htp = ab.enter_context(tc.tile_pool(name="hT", bufs=3))
            kst = ab.enter_context(tc.tile_pool(name="kstage", bufs=3))
            vst = ab.enter_context(tc.tile_pool(name="vstage", bufs=4))
            ptp = ab.enter_context(
                tc.tile_pool(name="ptrans", bufs=2, space="PSUM"))
            pqk = ab.enter_context(
                tc.tile_pool(name="pqk", bufs=2, space="PSUM"))
            pv = ab.enter_context(
                tc.tile_pool(name="pv", bufs=2, space="PSUM"))

            def ln_transpose(c, hT):
                c4 = c % 4
                if c in x_pre:
                    xt = x_pre.pop(c)
                else:
                    xt = xin.tile([P, D], F32, tag="x0", name=f"x{c}")
                    nc.sync.dma_start(
                        out=xt[:], in_=x_d[c * P:(c + 1) * P, :])
                st = stp.tile([P, 2, 6], F32, name=f"st{c}")
                nc.vector.bn_stats(out=st[:, 0, :], in_=xt[:, 0:512])
                nc.vector.bn_stats(out=st[:, 1, :], in_=xt[:, 512:D])
                mv_t = stp.tile([P, 2], F32, name=f"mv{c}", tag="mv")
                nc.vector.bn_aggr(out=mv_t[:], in_=st[:])
                rstd = stp.tile([P, 1], F32, name=f"rstd{c}", tag="rstd")
                nc.scalar.activation(
                    out=rstd[:], in_=mv_t[:, 1:2], func=AF.Sqrt,
                    bias=eps_t[:], scale=1.0)
                nc.vector.reciprocal(out=rstd[:], in_=rstd[:])
                nmr = stp.tile([P, 1], F32, name=f"nmr{c}", tag="nmr")
                nc.vector.tensor_scalar(
                    out=nmr[:], in0=mv_t[:, 0:1], scalar1=rstd[:],
                    scalar2=-1.0, op0=ALU.mult, op1=ALU.mult)
                ht_ = hp.tile([P, D], BF16, name=f"h{c}", tag="h")
                nc.vector.tensor_scalar(
                    out=ht_[:], in0=xt[:], scalar1=rstd[:],
                    scalar2=nmr[:], op0=ALU.mult, op1=ALU.add)
                for jh in range(2):
                    pt = ptp.tile([P, 512], F32, name=f"pt{c}_{jh}",
                                  tag="pt")
                    for jj in range(4):
                        j = jh * 4 + jj
                        nc.tensor.matmul(
                            pt[:, jj * P:(jj + 1) * P],
                            ht_[:, j * P:(j + 1) * P], ident[:],
                            start=True, stop=True)
                    nc.scalar.copy(
                        out=hT[:, jh * 4:(jh + 1) * 4,
                               c4 * P:(c4 + 1) * P],
                        in_=pt[:].rearrange("p (j c) -> p j c", j=4))

            def m1a(c, hT):
                c4 = c % 4
                pq = pqk.tile([P, D], F32, tag="pqk", name=f"pq{c}")
                for kt in range(NKT):
                    lhs = hT[:, kt, c4 * P:(c4 + 1) * P]
                    for nn_ in range(2):
                        nc.tensor.matmul(
                            pq[:, nn_ * 512:(nn_ + 1) * 512], lhs,
                            wqk[:, kt, nn_ * 512:(nn_ + 1) * 512],
                            start=(kt == 0), stop=(kt == NKT - 1))
                if zero_bias:
                    nc.vector.tensor_copy(out=q_sb[:, c, :], in_=pq[:])
                else:
                    nc.vector.tensor_tensor(
                        out=q_sb[:, c, :], in0=pq[:],
                        in1=biasqk[:, 0:D], op=ALU.add)
                pk = pqk.tile([P, D], F32, tag="pqk", name=f"pk{c}")
                for kt in range(NKT):
                    lhs = hT[:, kt, c4 * P:(c4 + 1) * P]
                    for nn_ in range(2):
                        nc.tensor.matmul(
                            pk[:, nn_ * 512:(nn_ + 1) * 512], lhs,
                            wqk[:, kt, D + nn_ * 512:D + (nn_ + 1) * 512],
                            start=(kt == 0), stop=(kt == NKT - 1))
                kt_stage = kst.tile([P, D], BF16, name=f"kst{c}", tag="kst")
                if zero_bias:
                    nc.vector.tensor_copy(out=kt_stage[:], in_=pk[:])
                else:
                    nc.vector.tensor_tensor(
                        out=kt_stage[:], in0=pk[:],
                        in1=biasqk[:, D:2 * D], op=ALU.add)
                nc.sync.dma_start(
                    out=k_spill[c * P:(c + 1) * P, :], in_=kt_stage[:])

            def m1b(g, hT):
                for mv in range(NKT):
                    pvt = pv.tile([P, 512], F32, name=f"pv{g}_{mv}",
                                  tag="pv")
                    for kt in range(NKT):
                        nc.tensor.matmul(
                            pvt[:], wv[:, kt, mv * P:(mv + 1) * P],
                            hT[:, kt, :], start=(kt == 0),
                            stop=(kt == NKT - 1))
                    v_stage = vst.tile([P, 512], BF16, name=f"vst{g}_{mv}",
                                       tag="vst")
                    if zero_bias:
                        if mv % 2 == 0:
                            nc.vector.tensor_copy(out=v_stage[:], in_=pvt[:])
                        else:
                            nc.scalar.copy(out=v_stage[:], in_=pvt[:])
                    else:
                        nc.vector.tensor_scalar_add(
                            out=v_stage[:], in0=pvt[:],
                            scalar1=biasv[:, mv:mv + 1])
                    nc.sync.dma_start(
                        out=vt_spill[mv * P:(mv + 1) * P,
                                     g * 512:(g + 1) * 512],
                        in_=v_stage[:])

            SKEW = 2
            hT_tiles = {}
            for c in range(NL + SKEW):
                if c < NL:
                    g = c // 4
                    if c % 4 == 0:
                        hT_tiles[g] = htp.tile([P, NKT, 512], BF16,
                                               name=f"hT{g}", tag="hT")
                    ln_transpose(c, hT_tiles[g])
                if c >= SKEW:
                    cp = c - SKEW
                    gp = cp // 4
                    m1a(cp, hT_tiles[gp])
                    if cp % 4 == 3:
                        m1b(gp, hT_tiles.pop(gp))

        nc.sync.dma_start(
            out=proj_sb[:], in_=proj_d.rearrange("(kt p) n -> p kt n", p=P))
        cdw = octx.enter_context(tc.tile_pool(name="cdw", bufs=1))
        w_sb = cdw.tile([P, NKT, D], BF16)
        wt_sb = cdw.tile([P, NKT, D], BF16)
        vtp = octx.enter_context(tc.tile_pool(name="vt", bufs=3))
        vt_tiles = {}
        vt_view = vt_spill.rearrange("(kt p) l -> p kt l", p=P)

        def load_vt(g):
            vt_tiles[g] = vtp.tile([P, NKT, 512], BF16, tag="vt",
                                   name=f"vt{g}")
            nc.sync.dma_start(
                out=vt_tiles[g][:],
                in_=vt_view[:, :, g * 512:(g + 1) * 512])

        # ---------------- Phase C: S = q^T k, softmax, transpose -------
        with ExitStack() as cc:
            kstr = cc.enter_context(tc.tile_pool(name="kstream", bufs=10))
            k_pre = {}
            for c in range(min(6, NL)):
                k_pre[c] = kstr.tile([P, D], BF16, tag="ks", name=f"kp{c}")
                nc.sync.dma_start(
                    out=k_pre[c][:], in_=k_spill[c * P:(c + 1) * P, :])
            ps = cc.enter_context(
                tc.tile_pool(name="ps", bufs=3, space="PSUM"))
            pwt = cc.enter_context(
                tc.tile_pool(name="pwt", bufs=2, space="PSUM"))
            sxp = cc.enter_context(tc.tile_pool(name="sxp", bufs=4))
            for pass_i, mqs in enumerate(([0, 1], [2, 3, 4], [5, 6, 7])):
                s_tiles = {mq: ps.tile([P, D], F32, tag="s", name=f"s{mq}")
                           for mq in mqs}
                for c in range(NL):
                    if pass_i == 0 and c in k_pre:
                        kt_t = k_pre.pop(c)
                    else:
                        kt_t = kstr.tile([P, D], BF16, tag="ks",
                                         name=f"ks{pass_i}_{c}")
                        nc.sync.dma_start(
                            out=kt_t[:], in_=k_spill[c * P:(c + 1) * P, :])
                    for mq in mqs:
                        lhs = q_sb[:, c, mq * P:(mq + 1) * P]
                        for nn_ in range(2):
                            nc.tensor.matmul(
                                s_tiles[mq][:, nn_ * 512:(nn_ + 1) * 512],
                                lhs, kt_t[:, nn_ * 512:(nn_ + 1) * 512],
                                start=(c == 0), stop=(c == NL - 1))
                if pass_i < min(2, NG) and pass_i not in vt_tiles:
                    load_vt(pass_i)
                for mq in mqs:
                    s_ps = s_tiles[mq]
                    sumexp = sxp.tile([P, 1], F32, name=f"se{mq}", tag="se")
                    nc.scalar.activation(
                        out=w_sb[:, mq, :], in_=s_ps[:], func=AF.Exp,
                        bias=0.0, scale=1.0, accum_out=sumexp[:])
                    nc.vector.reciprocal(
                        out=rs_sb[:, mq:mq + 1], in_=sumexp[:])
                    for jh in range(2):
                        pt = pwt.tile([P, 512], F32)
                        for jj in range(4):
                            j = jh * 4 + jj
                            nc.tensor.matmul(
                                pt[:, jj * P:(jj + 1) * P],
                                w_sb[:, mq, j * P:(j + 1) * P], ident[:],
                                start=True, stop=True)
                        nc.vector.tensor_copy(
                            out=wt_sb[:, jh * 4:(jh + 1) * 4,
                                      mq * P:(mq + 1) * P],
                            in_=pt[:].rearrange("p (j c) -> p j c", j=4))

        # ------------- Phase D+E: A^T = wT.T vT ; out = A projT --------
        with ExitStack() as de:
            atp = de.enter_context(tc.tile_pool(name="at", bufs=3))
            xrp = de.enter_context(tc.tile_pool(name="xr", bufs=3))
            osp = de.enter_context(tc.tile_pool(name="ost", bufs=3))
            pat = de.enter_context(
                tc.tile_pool(name="pat", bufs=2, space="PSUM"))
            po = de.enter_context(
                tc.tile_pool(name="po", bufs=2, space="PSUM"))
            for g in range(NG):
                if g not in vt_tiles:
                    load_vt(g)
                vt_g = vt_tiles.pop(g)
                if g + 2 < NG:
                    load_vt(g + 2)
                at_g = atp.tile([P, NKT, 512], BF16)
                for mq in range(NKT):
                    a_ps = pat.tile([P, 512], F32)
                    for kt in range(NKT):
                        nc.tensor.matmul(
                            a_ps[:], wt_sb[:, kt, mq * P:(mq + 1) * P],
                            vt_g[:, kt, :], start=(kt == 0),
                            stop=(kt == NKT - 1))
                    nc.scalar.activation(
                        out=at_g[:, mq, :], in_=a_ps[:], func=AF.Identity,
                        scale=rs_sb[:, mq:mq + 1])
                for c4 in range(4):
                    c = g * 4 + c4
                    o_ps = po.tile([P, D], F32)
                    for kt in range(NKT):
                        lhs = at_g[:, kt, c4 * P:(c4 + 1) * P]
                        for nn_ in range(2):
                            nc.tensor.matmul(
                                o_ps[:, nn_ * 512:(nn_ + 1) * 512], lhs,
                                proj_sb[:, kt, nn_ * 512:(nn_ + 1) * 512],
                                start=(kt == 0), stop=(kt == NKT - 1))
                    xr = xrp.tile([P, D], F32)
                    nc.sync.dma_start(
                        out=xr[:], in_=xres_d[c * P:(c + 1) * P, :])
                    o_sb = osp.tile([P, D], F32)
                    nc.vector.tensor_add(out=o_sb[:], in0=o_ps[:], in1=xr[:])
                    nc.sync.dma_start(
                        out=out_d[c * P:(c + 1) * P, :], in_=o_sb[:])


def make_in_map_general(xb, qkv_w, qkv_b, norm_w, norm_b, proj_w, proj_b, L):
    scale = np.float32(1.0 / math.sqrt(L))
    qkv_w = np.asarray(qkv_w, np.float32)
    norm_w = np.asarray(norm_w, np.float32)
    norm_b = np.asarray(norm_b, np.float32)
    qkv_b = np.asarray(qkv_b, np.float32)
    wfold = qkv_w * norm_w[None, :]
    bias = (qkv_b + qkv_w @ norm_b).copy()
    wfold[D:2 * D] *= scale
    bias[D:2 * D] *= scale
    bf = ml_dtypes.bfloat16
    return {
        "x": np.ascontiguousarray(xb, np.float32),
        "xres": (np.asarray(xb, np.float32)
                 + np.asarray(proj_b, np.float32)[None, :]),
        "wqkT": np.ascontiguousarray(wfold[:2 * D].T).astype(bf),
        "wvT": np.ascontiguousarray(wfold[2 * D:].T).astype(bf),
        "projT": np.ascontiguousarray(
            np.asarray(proj_w, np.float32).T).astype(bf),
        "biasqk": bias[:2 * D].astype(np.float32),
        "biasv": bias[2 * D:].astype(np.float32),
        "ident": np.eye(P, dtype=bf),
    }


_CACHED = {}


def _get_program(key, builder, *args):
    if key not in _CACHED:
        _CACHED[key] = builder(*args)
    return _CACHED[key]


def kernel(x, norm_w, norm_b, qkv_w, qkv_b, proj_w, proj_b, _trace=False):
    from concourse.bass_utils import run_bass_kernel_spmd

    x = np.asarray(x, np.float32)
    B, L, D_ = x.shape
    assert D_ == D
    gram_ok = (not np.any(np.asarray(qkv_b))
               and not np.any(np.asarray(norm_b)))
    if gram_ok:
        nc = _get_program(("gram", L), build_program_gram, L)
        in_maps = [
            make_in_map_gram(x[b], qkv_w, norm_w, proj_w, proj_b)
            for b in range(B)
        ]
    else:
        in_maps = [
            make_in_map_general(x[b], qkv_w, qkv_b, norm_w, norm_b, proj_w,
                                proj_b, L)
            for b in range(B)
        ]
        zero_bias = not (np.any(in_maps[0]["biasqk"])
                         or np.any(in_maps[0]["biasv"]))
        nc = _get_program(("gen", L, zero_bias), build_program_general, L,
                          zero_bias)
    res = run_bass_kernel_spmd(nc, in_maps, core_ids=list(range(B)),
                               trace=_trace)
    out = np.stack([res.results[i]["out"] for i in range(B)]).astype(np.float32)
    if _trace:
        return out, res
    return out


# revision 15
# speedup vs baseline: 1.0163x; 1.0163x over previous
"""Trainium2 Bass kernel for an AttnBlock (LayerNorm -> qkv -> feature-axis
attention -> proj -> residual), sharded batch-parallel across 8 NeuronCores.

Self-contained: hardcodes shapes (B=8, L=4096, D=1024, H=1) and runs via
concourse run_bass_kernel_spmd on cores 0-7.

Math per batch element b (n = b since H == 1):
    h   = LayerNorm(x) * norm_w + norm_b
    qkv = h @ qkv_w.T + qkv_b            # [L, 3D]
    q, k, v = qkv[:, :D], qkv[:, D:2D], qkv[:, 2D:]
    S   = q.T @ (k / sqrt(L))            # [D, D]  (contract over L)
    W   = softmax(S, axis=1)
    A   = v @ W.T                        # [L, D]
    out = A @ proj_w.T + proj_b + x

Fast path (zero qkv/norm biases, which setup_inputs always produces): the
q/k/v GEMMs are algebraically collapsed through the Gram matrix G = h.T @ h:
    S   = Wq.T @ G @ Wk                  (no q,k materialization)
    out_attn = h @ (Wv @ (Wn.T @ projT)) (no v,A materialization; Wn = softmax)
This halves the matmul FLOPs (51.6 -> 25.6 GFLOP per core) and removes the
W-transpose and all HBM spills. All matmuls bf16 with fp32 PSUM accumulation.

Phases (per core):
    A:  stream x in L-chunks of 128: LN -> h (bf16, kept fully in SBUF),
        G = h.T h accumulated in PSUM in two 4-row-tile passes.
    B:  T1 = G @ Wk      (G used as lhsT via symmetry)
    C:  S = Wq.T @ T1; softmax per 128-row tile (exp w/ scale 1/sqrt(L),
        accum rowsum), Wn = W / rowsum (bf16)
    D:  MT = Wn.T @ projT
    E2: N = Wv @ MT
    F:  per L-chunk: PE-transpose h chunk, out = hT.T @ N + (x + proj_b)

A general path (the original direct kernel, handling nonzero biases) is kept
as fallback.
"""

import math
import re
from contextlib import ExitStack

import ml_dtypes
import numpy as np

import concourse.bass as bass
import concourse.mybir as mybir
import concourse.tile as tile
from concourse.vector_clock import ScopedClock, VectorClock

F32 = mybir.dt.float32
BF16 = mybir.dt.bfloat16
AF = mybir.ActivationFunctionType
ALU = mybir.AluOpType

P = 128
D = 1024
NKT = D // P  # 8 tiles over D
LN_EPS = 1e-5


def _vc_ticks(vc):
    return [int(s) for s in re.findall(r"\d+", repr(vc))]


def _patched_drain_and_barrier(self, tick_clock, wait_clock):
    # This walrus build rejects >1 sync wait on one CTRL instruction; split
    # the kernel-tail drain into one drain per busy logical processor.
    for proc, t in enumerate(_vc_ticks(tick_clock.global_clock)):
        if t <= 0:
            continue
        d = self.nc.sync.drain()
        sub = VectorClock()
        sub.require_at_least(proc, t)
        wait_clock.add_sem_waits(d.ins, ScopedClock({None: sub}))
    self.nc.all_engine_barrier()
    popped = self.nc._tile_sem_poison_stack.pop()
    assert popped is self._sem_poison
    self.nc.clear_and_free_semaphores(list(self.sems.allocated().values()))
    self.nc.all_engine_barrier()


tile.TileContext._drain_and_barrier = _patched_drain_and_barrier

# This walrus build rejects >1 sync wait on any instruction. Spill excess
# waits onto preceding single-wait NoOps on the same engine (program order
# on the engine stream makes the split equivalent).
_MAXW = 1
_orig_commit = tile.TileContext._commit_instruction


def _commit_capped(self, inst, lazy_reg_writes=True):
    si = getattr(inst, "sync_info", None)
    eng = getattr(inst, "engine", None)
    if (si is not None and si.on_wait and len(si.on_wait) > _MAXW
            and eng is not None and eng != mybir.EngineType.Unassigned):
        waits = list(si.on_wait)
        while len(waits) > _MAXW:
            chunk, waits = waits[:_MAXW], waits[_MAXW:]
            nop = mybir.InstNoOp(
                name=f"I-{self.nc.next_id()}",
                sync_info=mybir.SyncInfo(on_wait=chunk, on_update=[]),
                bass_nofuse=True,
                engine=eng,
            )
            _orig_commit(self, nop, lazy_reg_writes=False)
        inst.sync_info = mybir.SyncInfo(on_wait=waits, on_update=si.on_update)
    return _orig_commit(self, inst, lazy_reg_writes)


tile.TileContext._commit_instruction = _commit_capped


# --------------------------------------------------------------------------
# Gram-restructured fast path (zero qkv/norm biases)
# --------------------------------------------------------------------------

def build_program_gram(L):
    NL = L // P  # 32 L-chunks
    nc = bass.Bass("TRN2", target_bir_lowering=False, debug=False)

    x_d = nc.dram_tensor("x", [L, D], F32, kind="ExternalInput").ap()
    xres_d = nc.dram_tensor("xres", [L, D], F32, kind="ExternalInput").ap()
    wq_d = nc.dram_tensor("wqT", [D, D], BF16, kind="ExternalInput").ap()
    wk_d = nc.dram_tensor("wkT", [D, D], BF16, kind="ExternalInput").ap()
    wv_d = nc.dram_tensor("wvKD", [D, D], BF16, kind="ExternalInput").ap()
    proj_d = nc.dram_tensor("projT", [D, D], BF16, kind="ExternalInput").ap()
    ident_d = nc.dram_tensor("ident", [P, P], BF16, kind="ExternalInput").ap()
    out_d = nc.dram_tensor("out", [L, D], F32, kind="ExternalOutput").ap()

    with tile.TileContext(nc) as tc:
        _emit_gram(tc, L, NL, x_d, xres_d, wq_d, wk_d, wv_d, proj_d, ident_d,
                   out_d)
    return nc


def _emit_gram(tc, L, NL, x_d, xres_d, wq_d, wk_d, wv_d, proj_d, ident_d,
               out_d):
    nc = tc.nc
    inv_sqrt_l = float(1.0 / math.sqrt(L))

    with ExitStack() as octx:
        # The tile allocator is a per-space stack: pools must be released in
        # LIFO order of their first allocation, so persistent tiles are
        # allocated up-front ordered by (reverse) release time. Two scratch
        # buffers are double-used across phases (T1 then MT; G then Wn) to
        # fund a deep x-stream pipeline. Peak SBUF ~193KB/partition.
        const = octx.enter_context(tc.tile_pool(name="const", bufs=1))
        ident = const.tile([P, P], BF16)
        eps_t = const.tile([P, 1], F32)

        hpool = octx.enter_context(tc.tile_pool(name="h", bufs=1))
        h_sb = hpool.tile([P, NL, D], BF16)

        npl = octx.enter_context(tc.tile_pool(name="nsb", bufs=1))
        n_sb = npl.tile([P, NKT, D], BF16)

        sE2 = ExitStack()   # projT/wv weights, T1/MT scratch, BE psum
        sD = ExitStack()    # G/Wn scratch
        sC = ExitStack()    # wqk (wk then wq), softmax scalars
        sA = ExitStack()    # x stream, LN stats, G psum

        w2 = sE2.enter_context(tc.tile_pool(name="w2", bufs=1))
        proj_sb = w2.tile([P, NKT, D], BF16)
        wv_sb = w2.tile([P, NKT, D], BF16)
        s1p = sE2.enter_context(tc.tile_pool(name="scr1", bufs=1))
        scr1 = s1p.tile([P, NKT, D], BF16)      # T1 in B..C, MT in D..E2

        s2p = sD.enter_context(tc.tile_pool(name="scr2", bufs=1))
        scr2 = s2p.tile([P, NKT, D], BF16)      # G in A..B, Wn in C..D

        w1 = sC.enter_context(tc.tile_pool(name="w1", bufs=1))
        wqk_sb = w1.tile([P, NKT, D], BF16)     # Wk in B, Wq in C
        sxp = sC.enter_context(tc.tile_pool(name="sxp", bufs=4))

        NPRE = 10
        xin = sA.enter_context(tc.tile_pool(name="xin", bufs=NPRE))
        x_pre = {}
        for c in range(min(NPRE, NL)):
            x_pre[c] = xin.tile([P, D], F32, tag="x0", name=f"xp{c}")
        stp = sA.enter_context(tc.tile_pool(name="stats", bufs=8))

        # DMA emission order: x prefetch first (sync queue), weights behind
        # it on the idle gpsimd queue as per-tile contiguous copies (the
        # fused rearrange DMA costs ~5-6us of issue time on the sequencer).
        for c in range(min(NPRE, NL)):
            nc.sync.dma_start(out=x_pre[c][:], in_=x_d[c * P:(c + 1) * P, :])
        nc.vector.memset(eps_t[:], LN_EPS)
        nc.gpsimd.dma_start(out=ident[:], in_=ident_d[:])
        wk_view = wk_d.rearrange("(t p) n -> p t n", p=P)
        wq_view = wq_d.rearrange("(t p) n -> p t n", p=P)
        proj_view = proj_d.rearrange("(t p) n -> p t n", p=P)
        wv_view = wv_d.rearrange("(t p) n -> p t n", p=P)
        # Weight loads ride the gpsimd queue, which also carries the tiny
        # per-chunk nmr op: issue them INTERLEAVED with the chunk stream
        # (one per chunk) so they never head-of-line block the LN chain.
        # Wq is staged in n_sb's space (N is only written in E2, long after
        # phase C's last Wq read) - avoids a wk->wq reload seam before C.
        wdma_q = (
            [(wqk_sb, wk_view, t) for t in range(NKT)]
            + [(n_sb, wq_view, t) for t in range(NKT)]
            + [(proj_sb, proj_view, t) for t in range(NKT)]
            + [(wv_sb, wv_view, t) for t in range(NKT)]
        )

        # ---------------- Phase A: LN + h + G = h.T h ----------------
        # LN is split into front (stats, on DVE+scalar) and back (normalize)
        # halves, software-pipelined two chunks apart so DVE's in-order queue
        # never head-of-line blocks on the scalar sqrt. nmr rides GpSimd.
        def ln_front(c):
            if c in x_pre:
                xt = x_pre.pop(c)
            else:
                xt = xin.tile([P, D], F32, tag="x0", name=f"x{c}")
                nc.sync.dma_start(out=xt[:], in_=x_d[c * P:(c + 1) * P, :])
            st = stp.tile([P, 2, 6], F32, name=f"st{c}", tag="st")
            nc.vector.bn_stats(out=st[:, 0, :], in_=xt[:, 0:512])
            nc.vector.bn_stats(out=st[:, 1, :], in_=xt[:, 512:D])
            mv_t = stp.tile([P, 2], F32, name=f"mv{c}", tag="mv")
            nc.vector.bn_aggr(out=mv_t[:], in_=st[:])
            rstd = stp.tile([P, 1], F32, name=f"rstd{c}", tag="rstd")
            nc.scalar.activation(
                out=rstd[:], in_=mv_t[:, 1:2], func=AF.Sqrt,
                bias=eps_t[:], scale=1.0)
            return xt, mv_t, rstd

        def ln_back(c, xt, mv_t, rstd):
            nc.vector.reciprocal(out=rstd[:], in_=rstd[:])
            nmr = stp.tile([P, 1], F32, name=f"nmr{c}", tag="nmr")
            nc.vector.tensor_scalar(
                out=nmr[:], in0=mv_t[:, 0:1], scalar1=rstd[:],
                scalar2=-1.0, op0=ALU.mult, op1=ALU.mult)
            nc.scalar.activation(
                out=h_sb[:, c, :], in_=xt[:], func=AF.Identity,
                scale=rstd[:], bias=nmr[:])
            # drip one weight-tile DMA issue per chunk behind the nmr
            if wdma_q:
                dst, view, t = wdma_q.pop(0)
                nc.gpsimd.dma_start(out=dst[:, t, :], in_=view[:, t, :])
            if c >= NL - 4:
                while wdma_q:
                    dst, view, t = wdma_q.pop(0)
                    nc.gpsimd.dma_start(out=dst[:, t, :], in_=view[:, t, :])

        # G is symmetric: compute only the upper triangle (row-tile m covers
        # cols >= m*128) and fill the lower triangle with PE transposes.
        # Pass 0 (rows 0-3, 8 padded PSUM banks) overlaps the x stream;
        # pass 1 (rows 4-7, 4 banks) re-reads h from SBUF and overlaps the
        # lower-triangle fill-in.
        def g_row_mms(ptile, c, m):
            lhs = h_sb[:, c, m * P:(m + 1) * P]
            base = m * P
            rem = D - base
            off = 0
            while rem > 0:
                w = min(512, rem)
                nc.tensor.matmul(
                    ptile[:, off:off + w], lhs,
                    h_sb[:, c, base + off:base + off + w],
                    start=(c == 0), stop=(c == NL - 1))
                off += w
                rem -= w

        sGa = ExitStack()
        pGa = sGa.enter_context(tc.tile_pool(name="pGa", bufs=1, space="PSUM"))
        ptiles = [pGa.tile([P, D], F32, name=f"ga{m}", tag=f"g{m}")
                  for m in range(4)]
        pend = {}
        for c in range(min(4, NL)):
            pend[c] = ln_front(c)
        for c in range(NL):
            if c + 4 < NL:
                pend[c + 4] = ln_front(c + 4)
            ln_back(c, *pend.pop(c))
            for m in range(4):
                g_row_mms(ptiles[m], c, m)
        for m in range(4):
            nc.scalar.copy(out=scr2[:, m, m * P:D],
                           in_=ptiles[m][:, :D - m * P])
        sA.close()   # frees x-stream SBUF
        sGa.close()  # frees the 8 pass-0 PSUM banks

        sGb = ExitStack()
        pGb = sGb.enter_context(
            tc.tile_pool(name="pGb", bufs=1, space="PSUM"))
        pFill = sGb.enter_context(
            tc.tile_pool(name="pFill", bufs=2, space="PSUM"))

        def fill_row(r, cs):
            # scr2[:, r, c-slice] = transpose of scr2[:, c, r-slice]
            for c0 in range(0, len(cs), 4):
                grp = cs[c0:c0 + 4]
                pt = pFill.tile([P, 512], F32, name=f"fl{r}_{c0}", tag="fl")
                for j, c in enumerate(grp):
                    nc.tensor.matmul(
                        pt[:, j * P:(j + 1) * P],
                        scr2[:, c, r * P:(r + 1) * P], ident[:],
                        start=True, stop=True)
                nc.scalar.copy(
                    out=scr2[:, r, grp[0] * P:(grp[0] + len(grp)) * P],
                    in_=pt[:, :len(grp) * P].rearrange(
                        "p (j c2) -> p j c2", j=len(grp)))

        # pass 1 first (depends only on h), then the lower-triangle fills,
        # high columns first to match the descending-m T1 loop in phase B.
        gtiles = [pGb.tile([P, 512], F32, name=f"gb{m}", tag=f"gb{m}")
                  for m in range(4, 8)]
        for c in range(NL):
            for m in range(4, 8):
                g_row_mms(gtiles[m - 4], c, m)
        for m in range(4, 8):
            nc.scalar.copy(out=scr2[:, m, m * P:D],
                           in_=gtiles[m - 4][:, :D - m * P])
        for r in range(7, 4, -1):
            fill_row(r, list(range(4, r)))
        for r in range(7, 3, -1):
            fill_row(r, list(range(4)))
        for r in range(3, 0, -1):
            fill_row(r, list(range(r)))
        sGb.close()

        # ---------------- Phase B: T1 = G @ Wk ----------------
        psBE = sE2.enter_context(
            tc.tile_pool(name="psBE", bufs=3, space="PSUM"))

        for m in reversed(range(NKT)):
            pt1 = psBE.tile([P, D], F32, name=f"t1_{m}", tag="ps")
            for t in range(NKT):
                # lhsT = G[:, t, m-slice]: by symmetry of G this is the
                # [K=d2, M=d1] operand for row-block m of T1.
                lhs = scr2[:, t, m * P:(m + 1) * P]
                for nn in range(2):
                    nc.tensor.matmul(
                        pt1[:, nn * 512:(nn + 1) * 512], lhs,
                        wqk_sb[:, t, nn * 512:(nn + 1) * 512],
                        start=(t == 0), stop=(t == NKT - 1))
            nc.scalar.copy(out=scr1[:, m, :], in_=pt1[:])

        # ---------------- Phase C: S = Wq.T @ T1, softmax -> Wn --------
        for qs in range(NKT):
            ps = psBE.tile([P, D], F32, name=f"s_{qs}", tag="ps")
            for t in range(NKT):
                lhs = n_sb[:, t, qs * P:(qs + 1) * P]
                for nn in range(2):
                    nc.tensor.matmul(
                        ps[:, nn * 512:(nn + 1) * 512], lhs,
                        scr1[:, t, nn * 512:(nn + 1) * 512],
                        start=(t == 0), stop=(t == NKT - 1))
            # softmax (no max-subtraction: S = q.T k / sqrt(L) is O(5) for
            # normalized inputs; exp stays in fp32/bf16 range). exp writes
            # the G/Wn scratch directly; normalization is an in-place scale.
            sumexp = sxp.tile([P, 1], F32, name=f"se{qs}", tag="se")
            nc.scalar.activation(
                out=scr2[:, qs, :], in_=ps[:], func=AF.Exp, bias=0.0,
                scale=inv_sqrt_l, accum_out=sumexp[:])
            rsr = sxp.tile([P, 1], F32, name=f"rs{qs}", tag="rs")
            nc.vector.reciprocal(out=rsr[:], in_=sumexp[:])
            nc.vector.tensor_scalar_mul(
                out=scr2[:, qs, :], in0=scr2[:, qs, :], scalar1=rsr[:])

        sC.close()

        # ---------------- Phase D: MT = Wn.T @ projT ----------------
        for kt in range(NKT):
            pmt = psBE.tile([P, D], F32, name=f"mt_{kt}", tag="ps")
            for qs in range(NKT):
                lhs = scr2[:, qs, kt * P:(kt + 1) * P]
                for nn in range(2):
                    nc.tensor.matmul(
                        pmt[:, nn * 512:(nn + 1) * 512], lhs,
                        proj_sb[:, qs, nn * 512:(nn + 1) * 512],
                        start=(qs == 0), stop=(qs == NKT - 1))
            nc.scalar.copy(out=scr1[:, kt, :], in_=pmt[:])

        sD.close()

        # ---------------- Phase E2: N = Wv @ MT ----------------
        for dt in range(NKT):
            pn = psBE.tile([P, D], F32, name=f"n_{dt}", tag="ps")
            for kt in range(NKT):
                lhs = wv_sb[:, kt, dt * P:(dt + 1) * P]
                for nn in range(2):
                    nc.tensor.matmul(
                        pn[:, nn * 512:(nn + 1) * 512], lhs,
                        scr1[:, kt, nn * 512:(nn + 1) * 512],
                        start=(kt == 0), stop=(kt == NKT - 1))
            nc.scalar.copy(out=n_sb[:, dt, :], in_=pn[:])

        sE2.close()

        # ---------------- Phase F: out = h @ N + xres ----------------
        # Software-pipelined: chunk c+1's PE transposes are emitted before
        # chunk c's out matmuls so the PE never waits on the hT copy.
        htp = octx.enter_context(tc.tile_pool(name="hTc", bufs=3))
        ptp = octx.enter_context(
            tc.tile_pool(name="ptrans", bufs=2, space="PSUM"))
        pop = octx.enter_context(tc.tile_pool(name="po", bufs=2, space="PSUM"))
        xrp = octx.enter_context(tc.tile_pool(name="xr", bufs=4))
        osp = octx.enter_context(tc.tile_pool(name="ost", bufs=3))

        def transpose_chunk(c):
            xr = xrp.tile([P, D], F32, name=f"xr{c}", tag="xr")
            nc.sync.dma_start(out=xr[:], in_=xres_d[c * P:(c + 1) * P, :])
            hTc = htp.tile([P, NKT, P], BF16, name=f"hT{c}", tag="hT")
            for jh in range(2):
                pt = ptp.tile([P, 512], F32, name=f"pt{c}_{jh}", tag="pt")
                for jj in range(4):
                    j = jh * 4 + jj
                    nc.tensor.matmul(
                        pt[:, jj * P:(jj + 1) * P],
                        h_sb[:, c, j * P:(j + 1) * P], ident[:],
                        start=True, stop=True)
                nc.scalar.copy(
                    out=hTc[:, jh * 4:(jh + 1) * 4, :],
                    in_=pt[:].rearrange("p (j c2) -> p j c2", j=4))
            return xr, hTc

        cur = transpose_chunk(0)
        for c in range(NL):
            nxt = transpose_chunk(c + 1) if c + 1 < NL else None
            xr, hTc = cur
            po = pop.tile([P, D], F32, name=f"po{c}", tag="po")
            for t in range(NKT):
                for nn in range(2):
                    nc.tensor.matmul(
                        po[:, nn * 512:(nn + 1) * 512], hTc[:, t, :],
                        n_sb[:, t, nn * 512:(nn + 1) * 512],
                        start=(t == 0), stop=(t == NKT - 1))
            o_sb = osp.tile([P, D], F32, name=f"o{c}", tag="o")
            nc.vector.tensor_add(out=o_sb[:], in0=po[:], in1=xr[:])
            nc.sync.dma_start(
                out=out_d[c * P:(c + 1) * P, :], in_=o_sb[:])
            cur = nxt


def make_in_map_gram(xb, qkv_w, norm_w, proj_w, proj_b):
    qkv_w = np.asarray(qkv_w, np.float32)
    norm_w = np.asarray(norm_w, np.float32)
    wfold = qkv_w * norm_w[None, :]
    bf = ml_dtypes.bfloat16
    return {
        "x": np.ascontiguousarray(xb, np.float32),
        "xres": (np.asarray(xb, np.float32)
                 + np.asarray(proj_b, np.float32)[None, :]),
        "wqT": np.ascontiguousarray(wfold[:D].T).astype(bf),
        "wkT": np.ascontiguousarray(wfold[D:2 * D].T).astype(bf),
        "wvKD": np.ascontiguousarray(wfold[2 * D:]).astype(bf),
        "projT": np.ascontiguousarray(
            np.asarray(proj_w, np.float32).T).astype(bf),
        "ident": np.eye(P, dtype=bf),
    }


# --------------------------------------------------------------------------
# General fallback path (nonzero qkv/norm biases): original direct kernel
# --------------------------------------------------------------------------

def build_program_general(L, zero_bias=True):
    NL = L // P  # number of 128-row L chunks
    NG = L // 512  # number of 512-row L groups
    nc = bass.Bass("TRN2", target_bir_lowering=False, debug=False)

    x_d = nc.dram_tensor("x", [L, D], F32, kind="ExternalInput").ap()
    xres_d = nc.dram_tensor("xres", [L, D], F32, kind="ExternalInput").ap()
    wqk_d = nc.dram_tensor("wqkT", [D, 2 * D], BF16, kind="ExternalInput").ap()
    wv_d = nc.dram_tensor("wvT", [D, D], BF16, kind="ExternalInput").ap()
    proj_d = nc.dram_tensor("projT", [D, D], BF16, kind="ExternalInput").ap()
    biasqk_d = nc.dram_tensor("biasqk", [2 * D], F32, kind="ExternalInput").ap()
    biasv_d = nc.dram_tensor("biasv", [D], F32, kind="ExternalInput").ap()
    ident_d = nc.dram_tensor("ident", [P, P], BF16, kind="ExternalInput").ap()
    out_d = nc.dram_tensor("out", [L, D], F32, kind="ExternalOutput").ap()

    k_spill = nc.dram_tensor("k_spill", [L, D], BF16).ap()
    vt_spill = nc.dram_tensor("vt_spill", [D, L], BF16).ap()

    with tile.TileContext(nc) as tc:
        _emit_general(tc, L, NL, NG, x_d, xres_d, wqk_d, wv_d, proj_d,
                      biasqk_d, biasv_d, ident_d, out_d, k_spill, vt_spill,
                      zero_bias)
    return nc


def _emit_general(tc, L, NL, NG, x_d, xres_d, wqk_d, wv_d, proj_d, biasqk_d,
                  biasv_d, ident_d, out_d, k_spill, vt_spill, zero_bias):
    nc = tc.nc

    with ExitStack() as octx:
        const = octx.enter_context(tc.tile_pool(name="const", bufs=1))
        ident = const.tile([P, P], BF16)
        nc.sync.dma_start(out=ident[:], in_=ident_d[:])
        eps_t = const.tile([P, 1], F32)
        nc.vector.memset(eps_t[:], LN_EPS)
        proj_sb = const.tile([P, NKT, D], BF16)
        if not zero_bias:
            biasqk = const.tile([P, 2 * D], F32)
            nc.sync.dma_start(
                out=biasqk[:], in_=biasqk_d[None, :].to_broadcast((P, 2 * D)))
            biasv = const.tile([P, NKT], F32)
            nc.sync.dma_start(
                out=biasv[:], in_=biasv_d.rearrange("(mv p) -> p mv", p=P))
        # per-q-tile softmax 1/rowsum, filled in phase C, consumed in D
        rs_sb = const.tile([P, NKT], F32)

        qpool = octx.enter_context(tc.tile_pool(name="qres", bufs=1))
        q_sb = qpool.tile([P, NL, D], BF16)

        # ---------------- Phase AB: LN + qkv projection ----------------
        with ExitStack() as ab:
            xin = ab.enter_context(tc.tile_pool(name="xin", bufs=6))
            x_pre = {}
            for c in range(min(6, NL)):
                x_pre[c] = xin.tile([P, D], F32, tag="x0", name=f"xp{c}")
                nc.sync.dma_start(out=x_pre[c][:],
                                  in_=x_d[c * P:(c + 1) * P, :])

            abw = ab.enter_context(tc.tile_pool(name="abw", bufs=1))
            wqk = abw.tile([P, NKT, 2 * D], BF16)
            wqk_view = wqk_d.rearrange("(kt p) n -> p kt n", p=P)
            for kt in range(NKT):
                nc.sync.dma_start(out=wqk[:, kt, :], in_=wqk_view[:, kt, :])
            wv = abw.tile([P, NKT, D], BF16)
            nc.sync.dma_start(
                out=wv[:], in_=wv_d.rearrange("(kt p) n -> p kt n", p=P))

            stp = ab.enter_context(tc.tile_pool(name="stats", bufs=3))
            hp = ab.enter_context(tc.tile_pool(name="h", bufs=3))
            htp = ab.enter_context(tc.tile_pool(name="hT", bufs=3))
            kst = ab.enter_context(tc.tile_pool(name="kstage", bufs=3))
            vst = ab.enter_context(tc.tile_pool(name="vstage", bufs=4))
            ptp = ab.enter_context(
                tc.tile_pool(name="ptrans", bufs=2, space="PSUM"))
            pqk = ab.enter_context(
                tc.tile_pool(name="pqk", bufs=2, space="PSUM"))
            pv = ab.enter_context(
                tc.tile_pool(name="pv", bufs=2, space="PSUM"))

            def ln_transpose(c, hT):
                c4 = c % 4
                if c in x_pre:
                    xt = x_pre.pop(c)
                else:
                    xt = xin.tile([P, D], F32, tag="x0", name=f"x{c}")
                    nc.sync.dma_start(
                        out=xt[:], in_=x_d[c * P:(c + 1) * P, :])
                st = stp.tile([P, 2, 6], F32, name=f"st{c}")
                nc.vector.bn_stats(out=st[:, 0, :], in_=xt[:, 0:512])
                nc.vector.bn_stats(out=st[:, 1, :], in_=xt[:, 512:D])
                mv_t = stp.tile([P, 2], F32, name=f"mv{c}", tag="mv")
                nc.vector.bn_aggr(out=mv_t[:], in_=st[:])
                rstd = stp.tile([P, 1], F32, name=f"rstd{c}", tag="rstd")
                nc.scalar.activation(
                    out=rstd[:], in_=mv_t[:, 1:2], func=AF.Sqrt,
                    bias=eps_t[:], scale=1.0)
                nc.vector.reciprocal(out=rstd[:], in_=rstd[:])
                nmr = stp.tile([P, 1], F32, name=f"nmr{c}", tag="nmr")
                nc.vector.tensor_scalar(
                    out=nmr[:], in0=mv_t[:, 0:1], scalar1=rstd[:],
                    scalar2=-1.0, op0=ALU.mult, op1=ALU.mult)
                ht_ = hp.tile([P, D], BF16, name=f"h{c}", tag="h")
                nc.vector.tensor_scalar(
                    out=ht_[:], in0=xt[:], scalar1=rstd[:],
                    scalar2=nmr[:], op0=ALU.mult, op1=ALU.add)
                for jh in range(2):
                    pt = ptp.tile([P, 512], F32, name=f"pt{c}_{jh}",
                                  tag="pt")
                    for jj in range(4):
                        j = jh * 4 + jj
                        nc.tensor.matmul(
                            pt[:, jj * P:(jj + 1) * P],
                            ht_[:, j * P:(j + 1) * P], ident[:],
                            start=True, stop=True)
                    nc.scalar.copy(
                        out=hT[:, jh * 4:(jh + 1) * 4,
                               c4 * P:(c4 + 1) * P],
                        in_=pt[:].rearrange("p (j c) -> p j c", j=4))

            def m1a(c, hT):
                c4 = c % 4
                pq = pqk.tile([P, D], F32, tag="pqk", name=f"pq{c}")
                for kt in range(NKT):
                    lhs = hT[:, kt, c4 * P:(c4 + 1) * P]
                    for nn_ in range(2):
                        nc.tensor.matmul(
                            pq[:, nn_ * 512:(nn_ + 1) * 512], lhs,
                            wqk[:, kt, nn_ * 512:(nn_ + 1) * 512],
                            start=(kt == 0), stop=(kt == NKT - 1))
                if zero_bias:
                    nc.vector.tensor_copy(out=q_sb[:, c, :], in_=pq[:])
                else:
                    nc.vector.tensor_tensor(
                        out=q_sb[:, c, :], in0=pq[:],
                        in1=biasqk[:, 0:D], op=ALU.add)
                pk = pqk.tile([P, D], F32, tag="pqk", name=f"pk{c}")
                for kt in range(NKT):
                    lhs = hT[:, kt, c4 * P:(c4 + 1) * P]
                    for nn_ in range(2):
                        nc.tensor.matmul(
                            pk[:, nn_ * 512:(nn_ + 1) * 512], lhs,
                            wqk[:, kt, D + nn_ * 512:D + (nn_ + 1) * 512],
                            start=(kt == 0), stop=(kt == NKT - 1))
                kt_stage = kst.tile([P, D], BF16, name=f"kst{c}", tag="kst")
                if zero_bias:
                    nc.vector.tensor_copy(out=kt_stage[:], in_=pk[:])
                else:
                    nc.vector.tensor_tensor(
                        out=kt_stage[:], in0=pk[:],
                        in1=biasqk[:, D:2 * D], op=ALU.add)
                nc.sync.dma_start(
                    out=k_spill[c * P:(c + 1) * P, :], in_=kt_stage[:])

            def m1b(g, hT):
                for mv in range(NKT):
                    pvt = pv.tile([P, 512], F32, name=f"pv{g}_{mv}",
                                  tag="pv")
                    for kt in range(NKT):
                        nc.tensor.matmul(
                            pvt[:], wv[:, kt, mv * P:(mv + 1) * P],
                            hT[:, kt, :], start=(kt == 0),
                            stop=(kt == NKT - 1))
                    v_stage = vst.tile([P, 512], BF16, name=f"vst{g}_{mv}",
                                       tag="vst")
                    if zero_bias:
                        if mv % 2 == 0:
                            nc.vector.tensor_copy(out=v_stage[:], in_=pvt[:])
                        else:
                            nc.scalar.copy(out=v_stage[:], in_=pvt[:])
                    else:
                        nc.vector.tensor_scalar_add(
                            out=v_stage[:], in0=pvt[:],
                            scalar1=biasv[:, mv:mv + 1])
                    nc.sync.dma_start(
                        out=vt_spill[mv * P:(mv + 1) * P,
                                     g * 512:(g + 1) * 512],
                        in_=v_stage[:])

            SKEW = 2
            hT_tiles = {}
            for c in range(NL + SKEW):
                if c < NL:
                    g = c // 4
                    if c % 4 == 0:
                        hT_tiles[g] = htp.tile([P, NKT, 512], BF16,
                                               name=f"hT{g}", tag="hT")
                    ln_transpose(c, hT_tiles[g])
                if c >= SKEW:
                    cp = c - SKEW
                    gp = cp // 4
                    m1a(cp, hT_tiles[gp])
                    if cp % 4 == 3:
                        m1b(gp, hT_tiles.pop(gp))

        nc.sync.dma_start(
            out=proj_sb[:], in_=proj_d.rearrange("(kt p) n -> p kt n", p=P))
        cdw = octx.enter_context(tc.tile_pool(name="cdw", bufs=1))
        w_sb = cdw.tile([P, NKT, D], BF16)
        wt_sb = cdw.tile([P, NKT, D], BF16)
        vtp = octx.enter_context(tc.tile_pool(name="vt", bufs=3))
        vt_tiles = {}
        vt_view = vt_spill.rearrange("(kt p) l -> p kt l", p=P)

        def load_vt(g):
            vt_tiles[g] = vtp.tile([P, NKT, 512], BF16, tag="vt",
                                   name=f"vt{g}")
            nc.sync.dma_start(
                out=vt_tiles[g][:],
                in_=vt_view[:, :, g * 512:(g + 1) * 512])

        # ---------------- Phase C: S = q^T k, softmax, transpose -------
        with ExitStack() as cc:
            kstr = cc.enter_context(tc.tile_pool(name="kstream", bufs=10))
            k_pre = {}
            for c in range(min(6, NL)):
                k_pre[c] = kstr.tile([P, D], BF16, tag="ks", name=f"kp{c}")
                nc.sync.dma_start(
                    out=k_pre[c][:], in_=k_spill[c * P:(c + 1) * P, :])
            ps = cc.enter_context(
                tc.tile_pool(name="ps", bufs=3, space="PSUM"))
            pwt = cc.enter_context(
                tc.tile_pool(name="pwt", bufs=2, space="PSUM"))
            sxp = cc.enter_context(tc.tile_pool(name="sxp", bufs=4))
            for pass_i, mqs in enumerate(([0, 1], [2, 3, 4], [5, 6, 7])):
                s_tiles = {mq: ps.tile([P, D], F32, tag="s", name=f"s{mq}")
                           for mq in mqs}
                for c in range(NL):
                    if pass_i == 0 and c in k_pre:
                        kt_t = k_pre.pop(c)
                    else:
                        kt_t = kstr.tile([P, D], BF16, tag="ks",
                                         name=f"ks{pass_i}_{c}")
                        nc.sync.dma_start(
                            out=kt_t[:], in_=k_spill[c * P:(c + 1) * P, :])
                    for mq in mqs:
                        lhs = q_sb[:, c, mq * P:(mq + 1) * P]
                        for nn_ in range(2):
                            nc.tensor.matmul(
                                s_tiles[mq][:, nn_ * 512:(nn_ + 1) * 512],
                                lhs, kt_t[:, nn_ * 512:(nn_ + 1) * 512],
                                start=(c == 0), stop=(c == NL - 1))
                if pass_i < min(2, NG) and pass_i not in vt_tiles:
                    load_vt(pass_i)
                for mq in mqs:
                    s_ps = s_tiles[mq]
                    sumexp = sxp.tile([P, 1], F32, name=f"se{mq}", tag="se")
                    nc.scalar.activation(
                        out=w_sb[:, mq, :], in_=s_ps[:], func=AF.Exp,
                        bias=0.0, scale=1.0, accum_out=sumexp[:])
                    nc.vector.reciprocal(
                        out=rs_sb[:, mq:mq + 1], in_=sumexp[:])
                    for jh in range(2):
                        pt = pwt.tile([P, 512], F32)
                        for jj in range(4):
                            j = jh * 4 + jj
                            nc.tensor.matmul(
                                pt[:, jj * P:(jj + 1) * P],
                                w_sb[:, mq, j * P:(j + 1) * P], ident[:],
                                start=True, stop=True)
                        nc.vector.tensor_copy(
                            out=wt_sb[:, jh * 4:(jh + 1) * 4,
                                      mq * P:(mq + 1) * P],
                            in_=pt[:].rearrange("p (j c) -> p j c", j=4))

        # ------------- Phase D+E: A^T = wT.T vT ; out = A projT --------
        with ExitStack() as de:
            atp = de.enter_context(tc.tile_pool(name="at", bufs=3))
            xrp = de.enter_context(tc.tile_pool(name="xr", bufs=3))
            osp = de.enter_context(tc.tile_pool(name="ost", bufs=3))
            pat = de.enter_context(
                tc.tile_pool(name="pat", bufs=2, space="PSUM"))
            po = de.enter_context(
                tc.tile_pool(name="po", bufs=2, space="PSUM"))
            for g in range(NG):
                if g not in vt_tiles:
                    load_vt(g)
                vt_g = vt_tiles.pop(g)
                if g + 2 < NG:
                    load_vt(g + 2)
                at_g = atp.tile([P, NKT, 512], BF16)
                for mq in range(NKT):
                    a_ps = pat.tile([P, 512], F32)
                    for kt in range(NKT):
                        nc.tensor.matmul(
                            a_ps[:], wt_sb[:, kt, mq * P:(mq + 1) * P],
                            vt_g[:, kt, :], start=(kt == 0),
                            stop=(kt == NKT - 1))
                    nc.scalar.activation(
                        out=at_g[:, mq, :], in_=a_ps[:], func=AF.Identity,
                        scale=rs_sb[:, mq:mq + 1])
                for c4 in range(4):
                    c = g * 4 + c4
                    o_ps = po.tile([P, D], F32)
                    for kt in range(NKT):
                        lhs = at_g[:, kt, c4 * P:(c4 + 1) * P]
                        for nn_ in range(2):
                            nc.tensor.matmul(
                                o_ps[:, nn_ * 512:(nn_ + 1) * 512], lhs,
                                proj_sb[:, kt, nn_ * 512:(nn_ + 1) * 512],
                                start=(kt == 0), stop=(kt == NKT - 1))
                    xr = xrp.tile([P, D], F32)
                    nc.sync.dma_start(
                        out=xr[:], in_=xres_d[c * P:(c + 1) * P, :])
                    o_sb = osp.tile([P, D], F32)
                    nc.vector.tensor_add(out=o_sb[:], in0=o_ps[:], in1=xr[:])
                    nc.sync.dma_start(
                        out=out_d[c * P:(c + 1) * P, :], in_=o_sb[:])


def make_in_map_general(xb, qkv_w, qkv_b, norm_w, norm_b, proj_w, proj_b, L):
    scale = np.float32(1.0 / math.sqrt(L))
    qkv_w = np.asarray(qkv_w, np.float32)
    norm_w = np.asarray(norm_w, np.float32)
    norm_b = np.asarray(norm_b, np.float32)
    qkv_b = np.asarray(qkv_b, np.float32)
    wfold = qkv_w * norm_w[None, :]
    bias = (qkv_b + qkv_w @ norm_b).copy()
    wfold[D:2 * D] *= scale
    bias[D:2 * D] *= scale
    bf = ml_dtypes.bfloat16
    return {
        "x": np.ascontiguousarray(xb, np.float32),
        "xres": (np.asarray(xb, np.float32)
                 + np.asarray(proj_b, np.float32)[None, :]),
        "wqkT": np.ascontiguousarray(wfold[:2 * D].T).astype(bf),
        "wvT": np.ascontiguousarray(wfold[2 * D:].T).astype(bf),
        "projT": np.ascontiguousarray(
            np.asarray(proj_w, np.float32).T).astype(bf),
        "biasqk": bias[:2 * D].astype(np.float32),
        "biasv": bias[2 * D:].astype(np.float32),
        "ident": np.eye(P, dtype=bf),
    }


_CACHED = {}


def _get_program(key, builder, *args):
    if key not in _CACHED:
        _CACHED[key] = builder(*args)
    return _CACHED[key]


def kernel(x, norm_w, norm_b, qkv_w, qkv_b, proj_w, proj_b, _trace=False):
    from concourse.bass_utils import run_bass_kernel_spmd

    x = np.asarray(x, np.float32)
    B, L, D_ = x.shape
    assert D_ == D
    gram_ok = (not np.any(np.asarray(qkv_b))
               and not np.any(np.asarray(norm_b)))
    if gram_ok:
        nc = _get_program(("gram", L), build_program_gram, L)
        in_maps = [
            make_in_map_gram(x[b], qkv_w, norm_w, proj_w, proj_b)
            for b in range(B)
        ]
    else:
        in_maps = [
            make_in_map_general(x[b], qkv_w, qkv_b, norm_w, norm_b, proj_w,
                                proj_b, L)
            for b in range(B)
        ]
        zero_bias = not (np.any(in_maps[0]["biasqk"])
                         or np.any(in_maps[0]["biasv"]))
        nc = _get_program(("gen", L, zero_bias), build_program_general, L,
                          zero_bias)
    res = run_bass_kernel_spmd(nc, in_maps, core_ids=list(range(B)),
                               trace=_trace)
    out = np.stack([res.results[i]["out"] for i in range(B)]).astype(np.float32)
    if _trace:
        return out, res
    return out
